# revision 12
# baseline (speedup 1.0000x reference)
"""Trainium2 Bass kernel for MultiHeadFrequencyCrossAttention (v2).

Math (unchanged from v1): the fft2/ifft2 pipeline collapses exactly to
    energy = dk * D * Q @ K~^T      (K~ = K with head-dim index negated mod D)
so this is plain attention with a flipped-K and scale 512, done in fp16
hi/lo split precision (hi/lo stacking keeps it 2 matmuls per tile: cross
K=128 [lo;hi]x[hi;lo] + main K=65 [hi;-rowmax]x[hi;ones]).

v2 changes (scheduling, not math):
  * max-row transpose: the (128,8) per-block column maxes are moved into
    the (1,1024) fp16 bias row via 8 tiny PE matmuls against a fp16
    identity (colmax column as stationary), replacing v1's 4-byte-element
    DRAM bounce DMA (~10us dead time per head that also re-throttled the
    HAM clock gate to 4/8).
  * row-max reduction via DVE tensor_tensor_reduce over the two 512-col
    psum halves (max+min-of-negated fusion): ~2x cheaper than a 1024-col
    reduce_max.
  * normalize: 1/rowsums (ACT Ln + Exp(-x) -> fp16 row) is broadcast to
    64 partitions with a K=1 ones-matmul into PSUM, then one DVE multiply
    produces the fp16 Wo operand directly. No gpsimd custom ops anywhere
    (v1's partition_broadcast forced a gpsimd library load whose unload
    DRAIN sat ~8us on the critical tail).
  * emission order software-pipelines: maxS(h+1) matmuls run while head
    h's reduces/exp drain; transposes T(h) land one block after their
    reduces; the reciprocal/broadcast of head h hides inside main(h+1);
    AV matmuls lag their exp by 2 j-blocks so ACT never stalls PE.
  * output is fp16 (host up-casts and sums the two half-head partials).

Per-core fp16 output partials, summed on host across the 2 head-groups.
"""

import numpy as np
from contextlib import ExitStack

import concourse.bass as bass
import concourse.tile as tile
from concourse import bacc, mybir
from concourse.bass_utils import run_bass_kernel_spmd

F32 = mybir.dt.float32
F16 = mybir.dt.float16
BF16 = mybir.dt.bfloat16
AX = mybir.AxisListType
AF = mybir.ActivationFunctionType
ALU = mybir.AluOpType

T = 1024          # sequence length
E = 512           # embed dim
H = 8             # total heads
D = E // H        # head dim = 64
NH = 4            # heads per core
DX = NH * (D + 1) # vp columns incl. ones = 260
N_CORES = 8
SCALE = float(D) * float(D) ** 0.5  # dk * D = 512.0

TRACE = False
LAST_EXEC_NS = None


def _emit(ctx, tc, dram):
    nc = tc.nc
    const = ctx.enter_context(tc.tile_pool(name="const", bufs=1))
    ps = ctx.enter_context(tc.tile_pool(name="ps", bufs=1, space="PSUM"))
    atp = ctx.enter_context(tc.tile_pool(name="atp", bufs=6))
    outp = ctx.enter_context(tc.tile_pool(name="outp", bufs=1))

    # ---- input loads (one 3D DMA per matrix) ----
    def load1(name, cols):
        t3 = const.tile([128, 4, cols], F16, tag=name, name=name)
        nc.sync.dma_start(t3[:], dram[name][:])
        return [t3[:, e, :] for e in range(4)]

    wqh = load1("wqh", NH * D)
    ql_in = load1("ql", T)
    wql = load1("wql", NH * D)
    qh_in = load1("qh", T)
    wkh = load1("wkh", NH * D)
    kvl_in = load1("kvl", T)
    wkl = load1("wkl", NH * D)
    kvh_in = load1("kvh", T)
    wv = load1("wv", DX)
    wo3 = const.tile([128, 2, E], F16, tag="wo", name="wo")
    nc.sync.dma_start(wo3[:], dram["wo"][:])
    wo = [wo3[:, g, :] for g in range(2)]
    id128 = const.tile([128, 128], F16, tag="id", name="id")
    nc.sync.dma_start(id128[:], dram["id"][:])

    # PE warm-up: fills the input-DMA window so the HAM clock gate is at
    # 8/8 by the time the projections start.
    wrm = const.tile([128, 512], F16, tag="wrm", name="wrm")
    nc.vector.memset(wrm[:], 0.0)
    for _ in range(16):
        pw = ps.tile([128, E], F32, tag="big", bufs=3, name="pw")
        nc.tensor.matmul(pw[:], lhsT=wrm[:, 0:128], rhs=wrm[:],
                         start=True, stop=True)

    # ---- hi/lo projections ----
    qm = [const.tile([65, T], F16, tag=f"qm{h}", name=f"qm{h}") for h in range(NH)]
    km = [const.tile([65, T], F16, tag=f"km{h}", name=f"km{h}") for h in range(NH)]
    qc = [const.tile([128, T], F16, tag=f"qc{h}", name=f"qc{h}") for h in range(NH)]
    kc = [const.tile([128, T], F16, tag=f"kc{h}", name=f"kc{h}") for h in range(NH)]

    for wi, (wh, wl, xh, xl, dm, dc, hi_row) in enumerate((
        (wqh, wql, qh_in, ql_in, qm, qc, 64),   # qc rows: [lo; hi]
        (wkh, wkl, kvh_in, kvl_in, km, kc, 0),  # kc rows: [hi; lo]
    )):
        psb2 = [ps.tile([128, T], F32, tag="big", bufs=3, name=f"psb{m}")
                for m in range(2)]
        waves = ([(wh, xl)], [(wl, xh)], [(wh, xh)])
        for i_w, wave in enumerate(waves):
            for m in range(2):
                msl = slice(m * 128, (m + 1) * 128)
                for n in range(2):
                    nsl = slice(n * 512, (n + 1) * 512)
                    for lw4, rx4 in wave:
                        for e in range(4):
                            nc.tensor.matmul(
                                psb2[m][:, nsl],
                                lhsT=lw4[e][:, msl],
                                rhs=rx4[e][:, nsl],
                                start=(i_w == 0 and e == 0),
                                stop=(i_w == 2 and e == 3),
                            )
        for m in range(2):
            msl = slice(m * 128, (m + 1) * 128)
            psb = psb2[m]
            for hh in range(2):
                h = 2 * m + hh
                psl = slice(hh * 64, hh * 64 + 64)
                lo_row = 64 - hi_row
                # hi part (fp16 cast) into the K=65 "main" tile (ACT)
                nc.scalar.copy(dm[h][0:64, :], psb[psl, :])
                # hi copy into the cross tile (alternate ACT / DVE)
                nc.vector.tensor_copy(dc[h][hi_row:hi_row + 64, :],
                                      dm[h][0:64, :])
                # lo part = ps - hi (fp16) on DVE
                nc.vector.tensor_sub(dc[h][lo_row:lo_row + 64, :], psb[psl, :],
                                     dm[h][0:64, :])
    for h in range(NH):
        nc.gpsimd.memset(km[h][64:65, :], 1.0)

    # vp natural (t-major) + ones columns, bf16
    vpx = [const.tile([128, DX], BF16, tag=f"vpx{t}", name=f"vpx{t}")
           for t in range(8)]

    def emit_vp_maxS0():
        for t in range(8):
            maxS_block(0, t)
            psv = ps.tile([128, E], F32, tag="big", bufs=3, name="psv")
            for e in range(4):
                nc.tensor.matmul(
                    psv[:, 0:DX],
                    lhsT=kvh_in[e][:, t * 128:(t + 1) * 128],
                    rhs=wv[e][:],
                    start=(e == 0), stop=(e == 3),
                )
            nc.scalar.copy(vpx[t][:], psv[:, 0:DX])
            for h4 in range(NH):
                c = h4 * (D + 1) + D
                nc.gpsimd.memset(vpx[t][:, c:c + 1], 1.0)

    # ---- per-head attention state ----
    colmax = [const.tile([128, 8], F16, tag=f"cm{h}", name=f"cm{h}")
              for h in range(NH)]
    yun = [const.tile([64, T], F32, tag=f"yun{h}", name=f"yun{h}")
           for h in range(NH)]
    yh = [const.tile([128, T], F16, tag=f"yh{g}", name=f"yh{g}")
          for g in range(2)]
    lns = [const.tile([1, T], F32, tag=f"ln{h}", name=f"ln{h}")
           for h in range(NH)]
    F32R = mybir.dt.float32r
    recip = [const.tile([1, T], F32R, tag=f"rcp{h}", name=f"rcp{h}")
             for h in range(NH)]
    ones64 = const.tile([1, 64], F32R, tag="ones64", name="ones64")
    nc.sync.dma_start(ones64[:], dram["one64"][:])
    oex = [None] * NH  # per-head AV psum, allocated in mainpass

    def maxS_block(h, i):
        # one 128-row i-block of the fp16 hi-only max pass; DVE negated
        # row-max into colmax column i
        psb = ps.tile([128, T], F32, tag="big", bufs=3, name="psb")
        for n in range(2):
            nsl = slice(n * 512, (n + 1) * 512)
            nc.tensor.matmul(
                psb[:, nsl],
                lhsT=qm[h][0:64, i * 128:(i + 1) * 128],
                rhs=km[h][0:64, nsl],
                start=True, stop=True,
            )
        nc.vector.reduce_max(colmax[h][:, i:i + 1], psb[:], axis=AX.X,
                             negate=True)

    def trpT(h):
        # (128,8) negated col maxes -> (1,1024) fp16 bias row of qm[h],
        # via 8 single-column stationary matmuls against identity.
        for half in range(2):
            prow = ps.tile([1, 512], F32, tag="big", bufs=3, name="prow")
            for cc in range(4):
                c = half * 4 + cc
                nc.tensor.matmul(
                    prow[:, cc * 128:(cc + 1) * 128],
                    lhsT=colmax[h][:, c:c + 1], rhs=id128[:],
                    start=True, stop=True,
                )
            nc.scalar.copy(qm[h][64:65, half * 512:(half + 1) * 512], prow[:])

    def rest(h):
        # reciprocal broadcast + normalize -> fp16 Wo operand rows
        g, hhalf = divmod(h, 2)
        for n in range(2):
            nsl = slice(n * 512, (n + 1) * 512)
            prec = ps.tile([64, 512], F32, tag="big", bufs=3, name="prec")
            nc.tensor.matmul(prec[:], lhsT=ones64[:], rhs=recip[h][:, nsl],
                             start=True, stop=True)
            nc.vector.tensor_mul(
                yh[g][hhalf * 64:(hhalf + 1) * 64, nsl],
                yun[h][:, nsl], prec[:],
            )

    def mainpass(h, rest_h=None, maxs_h=None):
        # S^T - max = cross(K=128) + main(K=65 w/ bias row), exp, AV.
        # maxs_h's max-pass blocks are interleaved per j so the PE never
        # idles while DVE drains the reduce stream.
        oex[h] = ps.tile([65, T], F32, tag="av", name="oex")
        at = [None] * 8

        def av(j):
            for n in range(2):
                nsl = slice(n * 512, (n + 1) * 512)
                nc.tensor.matmul(
                    oex[h][:, nsl],
                    lhsT=vpx[j][:, h * (D + 1):(h + 1) * (D + 1)],
                    rhs=at[j][:, nsl],
                    start=(j == 0), stop=(j == 7),
                )

        for j in range(8):
            jsl = slice(j * 128, (j + 1) * 128)
            if maxs_h is not None:
                maxS_block(maxs_h, j)
            psb = ps.tile([128, T], F32, tag="big", bufs=3, name="psb")
            for n in range(2):
                nsl = slice(n * 512, (n + 1) * 512)
                nc.tensor.matmul(
                    psb[:, nsl], lhsT=kc[h][:, jsl], rhs=qc[h][:, nsl],
                    start=True, stop=False,
                )
                nc.tensor.matmul(
                    psb[:, nsl], lhsT=km[h][:, jsl], rhs=qm[h][:, nsl],
                    start=False, stop=True,
                )
            at[j] = atp.tile([128, T], BF16, tag="at", name="at")
            nc.scalar.activation(at[j][:], psb[:], AF.Exp)
            if j >= 2:
                av(j - 2)
            if j == 3 and rest_h is not None:
                rest(rest_h)
        av(6)
        av(7)
        # evacuate Y^T (raw) and the rowsums' reciprocal
        nc.vector.tensor_copy(yun[h][:], oex[h][0:64, :])
        nc.scalar.activation(lns[h][:], oex[h][64:65, :], AF.Ln)
        nc.scalar.activation(recip[h][:], lns[h][:], AF.Exp, scale=-1.0)

    def wo_pass(g):
        for i in range(8):
            pso = ps.tile([128, E], F32, tag="big", bufs=3, name="pso")
            nc.tensor.matmul(
                pso[:],
                lhsT=yh[g][:, i * 128:(i + 1) * 128],
                rhs=wo[g][:],
                start=True, stop=True,
            )
            o16 = outp.tile([128, E], F16, tag="o16", bufs=4, name="o16")
            if i % 2 == 0:
                nc.vector.tensor_copy(o16[:], pso[:])
            else:
                nc.scalar.copy(o16[:], pso[:])
            nc.sync.dma_start(dram[f"out{g}"][i * 128:(i + 1) * 128, :], o16[:])

    # ---- schedule ----
    emit_vp_maxS0()
    trpT(0)
    mainpass(0, maxs_h=1)
    trpT(1)
    mainpass(1, rest_h=0, maxs_h=2)
    trpT(2)
    mainpass(2, rest_h=1, maxs_h=3)
    trpT(3)
    wo_pass(0)
    mainpass(3, rest_h=2)
    rest(3)
    wo_pass(1)


class _Bacc(bacc.Bacc):
    """Bacc whose activation-table chooser can only pick the combined
    natural_log_exp_and_others set for Exp/Ln (removes per-head Exp<->Ln
    table swaps)."""

    def insert_act_table_loads(self):
        import bass_rust as _bass_rust
        from concourse.hw_specs import get_activation_tables
        has_activation = any(
            isinstance(i, mybir.InstActivation)
            for b in self.main_func.blocks
            for i in b.instructions
        )
        if not has_activation:
            return
        tables = []
        for name, fns in get_activation_tables(self.m.arch).items():
            if name in ("exp_and_others", "exp_and_friends"):
                fns = set()
            tables.append((name, fns))
        _bass_rust.insert_act_table_loads(self, tables)


def build_program():
    nc = _Bacc("TRN2", target_bir_lowering=False, debug=False)
    dp = nc.declare_dram_parameter
    dram = {}
    for name in ("qh", "ql", "kvh", "kvl"):
        dram[name] = dp(name, [128, 4, T], F16, isOutput=False)
    for name in ("wqh", "wql", "wkh", "wkl"):
        dram[name] = dp(name, [128, 4, NH * D], F16, isOutput=False)
    dram["wv"] = dp("wv", [128, 4, DX], F16, isOutput=False)
    dram["wo"] = dp("wo", [128, 2, E], F16, isOutput=False)
    dram["id"] = dp("id", [128, 128], F16, isOutput=False)
    dram["one64"] = dp("one64", [1, 64], mybir.dt.float32r, isOutput=False)
    dram["out0"] = dp("out0", [T, E], F16, isOutput=True)
    dram["out1"] = dp("out1", [T, E], F16, isOutput=True)
    with ExitStack() as ctx:
        tc = ctx.enter_context(tile.TileContext(nc))
        _emit(ctx, tc, dram)
    nc.finalize()
    return nc


_PROGRAM = None


def _get_program():
    global _PROGRAM
    if _PROGRAM is None:
        _PROGRAM = build_program()
    return _PROGRAM


def _split16(x):
    h = x.astype(np.float16)
    l = (x - h.astype(np.float32)).astype(np.float16)
    return h, l


def _pmajor(x, chunks):
    """(C*128, cols) row-major -> (128, C, cols): partition-major layout so
    the input DMA is one contiguous line per partition."""
    rows, cols = x.shape
    return np.ascontiguousarray(
        x.reshape(chunks, 128, cols).transpose(1, 0, 2))


_ID128 = np.eye(128, dtype=np.float16)


def make_in_maps(q, kv, Wq, Wk, Wv, Wo):
    in_maps = []
    for c in range(N_CORES):
        b, g = divmod(c, 2)
        heads = [g * NH + j for j in range(NH)]
        idx_q = [d * H + h for h in heads for d in range(D)]
        idx_k = [((D - d) % D) * H + h for h in heads for d in range(D)]
        qTh, qTl = _split16(np.ascontiguousarray(q[b].T))
        kvTh, kvTl = _split16(np.ascontiguousarray(kv[b].T))
        wq_h, wq_l = _split16(Wq[:, idx_q] * np.float32(SCALE))
        wk_h, wk_l = _split16(Wk[:, idx_k])
        wv_c = np.zeros((E, DX), np.float16)
        for j, h in enumerate(heads):
            wv_c[:, j * (D + 1):j * (D + 1) + D] = \
                Wv[:, [d * H + h for d in range(D)]].astype(np.float16)
        in_maps.append({
            "qh": _pmajor(qTh, 4), "ql": _pmajor(qTl, 4),
            "kvh": _pmajor(kvTh, 4), "kvl": _pmajor(kvTl, 4),
            "wqh": _pmajor(wq_h, 4), "wql": _pmajor(wq_l, 4),
            "wkh": _pmajor(wk_h, 4), "wkl": _pmajor(wk_l, 4),
            "wv": _pmajor(wv_c, 4),
            "wo": _pmajor(
                Wo[g * NH * D:(g + 1) * NH * D, :].astype(np.float16), 2),
            "id": _ID128,
            "one64": np.ones((1, 64), np.float32),
        })
    return in_maps


def kernel(**inputs):
    global LAST_EXEC_NS
    q = np.asarray(inputs["q"], dtype=np.float32)
    kv = np.asarray(inputs["kv"], dtype=np.float32)
    Wq = np.asarray(inputs["Wq"], dtype=np.float32)
    Wk = np.asarray(inputs["Wk"], dtype=np.float32)
    Wv = np.asarray(inputs["Wv"], dtype=np.float32)
    Wo = np.asarray(inputs["Wo"], dtype=np.float32)
    B = q.shape[0]

    nc = _get_program()
    in_maps = make_in_maps(q, kv, Wq, Wk, Wv, Wo)
    res = run_bass_kernel_spmd(nc, in_maps, list(range(N_CORES)), trace=TRACE)
    LAST_EXEC_NS = res.exec_time_ns

    out = np.empty((B, T, E), np.float32)
    for b in range(B):
        out[b] = (res.results[2 * b]["out0"].astype(np.float32)
                  + res.results[2 * b]["out1"].astype(np.float32)
                  + res.results[2 * b + 1]["out0"].astype(np.float32)
                  + res.results[2 * b + 1]["out1"].astype(np.float32))
    return out


# revision 13
# speedup vs baseline: 1.0404x; 1.0404x over previous
"""Trainium2 Bass kernel for MultiHeadFrequencyCrossAttention (v2).

Math (unchanged from v1): the fft2/ifft2 pipeline collapses exactly to
    energy = dk * D * Q @ K~^T      (K~ = K with head-dim index negated mod D)
so this is plain attention with a flipped-K and scale 512, done in fp16
hi/lo split precision (hi/lo stacking keeps it 2 matmuls per tile: cross
K=128 [lo;hi]x[hi;lo] + main K=65 [hi;-rowmax]x[hi;ones]).

v2 changes (scheduling, not math):
  * max-row transpose: the (128,8) per-block column maxes are moved into
    the (1,1024) fp16 bias row via 8 tiny PE matmuls against a fp16
    identity (colmax column as stationary), replacing v1's 4-byte-element
    DRAM bounce DMA (~10us dead time per head that also re-throttled the
    HAM clock gate to 4/8).
  * row-max reduction via DVE tensor_tensor_reduce over the two 512-col
    psum halves (max+min-of-negated fusion): ~2x cheaper than a 1024-col
    reduce_max.
  * normalize: 1/rowsums (ACT Ln + Exp(-x) -> fp16 row) is broadcast to
    64 partitions with a K=1 ones-matmul into PSUM, then one DVE multiply
    produces the fp16 Wo operand directly. No gpsimd custom ops anywhere
    (v1's partition_broadcast forced a gpsimd library load whose unload
    DRAIN sat ~8us on the critical tail).
  * emission order software-pipelines: maxS(h+1) matmuls run while head
    h's reduces/exp drain; transposes T(h) land one block after their
    reduces; the reciprocal/broadcast of head h hides inside main(h+1);
    AV matmuls lag their exp by 2 j-blocks so ACT never stalls PE.
  * output is fp16 (host up-casts and sums the two half-head partials).

Per-core fp16 output partials, summed on host across the 2 head-groups.
"""

import numpy as np
from contextlib import ExitStack

import concourse.bass as bass
import concourse.tile as tile
from concourse import bacc, mybir
from concourse.bass_utils import run_bass_kernel_spmd

F32 = mybir.dt.float32
F16 = mybir.dt.float16
BF16 = mybir.dt.bfloat16
AX = mybir.AxisListType
AF = mybir.ActivationFunctionType
ALU = mybir.AluOpType

T = 1024          # sequence length
E = 512           # embed dim
H = 8             # total heads
D = E // H        # head dim = 64
NH = 4            # heads per core
DX = NH * (D + 1) # vp columns incl. ones = 260
N_CORES = 8
SCALE = float(D) * float(D) ** 0.5  # dk * D = 512.0

TRACE = False
LAST_EXEC_NS = None


def _emit(ctx, tc, dram):
    nc = tc.nc
    const = ctx.enter_context(tc.tile_pool(name="const", bufs=1))
    ps = ctx.enter_context(tc.tile_pool(name="ps", bufs=1, space="PSUM"))
    atp = ctx.enter_context(tc.tile_pool(name="atp", bufs=6))
    outp = ctx.enter_context(tc.tile_pool(name="outp", bufs=1))

    # ---- input loads (one 3D DMA per matrix) ----
    def load1(name, cols):
        t3 = const.tile([128, 4, cols], F16, tag=name, name=name)
        nc.sync.dma_start(t3[:], dram[name][:])
        return [t3[:, e, :] for e in range(4)]

    wqh = load1("wqh", NH * D)
    ql_in = load1("ql", T)
    wql = load1("wql", NH * D)
    qh_in = load1("qh", T)
    wkh = load1("wkh", NH * D)
    kvl_in = load1("kvl", T)
    wkl = load1("wkl", NH * D)
    kvh_in = load1("kvh", T)
    wv = load1("wv", DX)
    wo3 = const.tile([128, 2, E], F16, tag="wo", name="wo")
    nc.sync.dma_start(wo3[:], dram["wo"][:])
    wo = [wo3[:, g, :] for g in range(2)]
    id128 = const.tile([128, 128], F16, tag="id", name="id")
    nc.sync.dma_start(id128[:], dram["id"][:])

    # PE warm-up: fills the input-DMA window so the HAM clock gate is at
    # 8/8 by the time the projections start.
    wrm = const.tile([128, 512], F16, tag="wrm", name="wrm")
    nc.vector.memset(wrm[:], 0.0)
    for _ in range(16):
        pw = ps.tile([128, E], F32, tag="big", bufs=3, name="pw")
        nc.tensor.matmul(pw[:], lhsT=wrm[:, 0:128], rhs=wrm[:],
                         start=True, stop=True)

    # ---- hi/lo projections ----
    qm = [const.tile([65, T], F16, tag=f"qm{h}", name=f"qm{h}") for h in range(NH)]
    km = [const.tile([65, T], F16, tag=f"km{h}", name=f"km{h}") for h in range(NH)]
    qc = [const.tile([128, T], F16, tag=f"qc{h}", name=f"qc{h}") for h in range(NH)]
    kc = [const.tile([128, T], F16, tag=f"kc{h}", name=f"kc{h}") for h in range(NH)]

    for wi, (wh, wl, xh, xl, dm, dc, hi_row) in enumerate((
        (wqh, wql, qh_in, ql_in, qm, qc, 64),   # qc rows: [lo; hi]
        (wkh, wkl, kvh_in, kvl_in, km, kc, 0),  # kc rows: [hi; lo]
    )):
        psb2 = [ps.tile([128, T], F32, tag="big", bufs=3, name=f"psb{m}")
                for m in range(2)]
        waves = ([(wh, xl)], [(wl, xh)], [(wh, xh)])
        for i_w, wave in enumerate(waves):
            for m in range(2):
                msl = slice(m * 128, (m + 1) * 128)
                for n in range(2):
                    nsl = slice(n * 512, (n + 1) * 512)
                    for lw4, rx4 in wave:
                        for e in range(4):
                            nc.tensor.matmul(
                                psb2[m][:, nsl],
                                lhsT=lw4[e][:, msl],
                                rhs=rx4[e][:, nsl],
                                start=(i_w == 0 and e == 0),
                                stop=(i_w == 2 and e == 3),
                            )
        for m in range(2):
            msl = slice(m * 128, (m + 1) * 128)
            psb = psb2[m]
            for hh in range(2):
                h = 2 * m + hh
                psl = slice(hh * 64, hh * 64 + 64)
                lo_row = 64 - hi_row
                # hi part (fp16 cast) into the K=65 "main" tile (ACT)
                nc.scalar.copy(dm[h][0:64, :], psb[psl, :])
                # hi copy into the cross tile (alternate ACT / DVE)
                nc.vector.tensor_copy(dc[h][hi_row:hi_row + 64, :],
                                      dm[h][0:64, :])
                # lo part = ps - hi (fp16) on DVE
                nc.vector.tensor_sub(dc[h][lo_row:lo_row + 64, :], psb[psl, :],
                                     dm[h][0:64, :])
    for h in range(NH):
        nc.gpsimd.memset(km[h][64:65, :], 1.0)

    # vp natural (t-major) + ones columns, bf16
    vpx = [const.tile([128, DX], BF16, tag=f"vpx{t}", name=f"vpx{t}")
           for t in range(8)]

    def emit_vp_maxS0():
        for t in range(8):
            maxS_block(0, t)
            psv = ps.tile([128, E], F32, tag="big", bufs=3, name="psv")
            for e in range(4):
                nc.tensor.matmul(
                    psv[:, 0:DX],
                    lhsT=kvh_in[e][:, t * 128:(t + 1) * 128],
                    rhs=wv[e][:],
                    start=(e == 0), stop=(e == 3),
                )
            nc.scalar.copy(vpx[t][:], psv[:, 0:DX])
            for h4 in range(NH):
                c = h4 * (D + 1) + D
                nc.gpsimd.memset(vpx[t][:, c:c + 1], 1.0)

    # ---- per-head attention state ----
    colmax = [const.tile([128, 8], F16, tag=f"cm{h}", name=f"cm{h}")
              for h in range(NH)]
    yun = [const.tile([64, T], F32, tag=f"yun{h}", name=f"yun{h}")
           for h in range(NH)]
    yh = [const.tile([128, T], F16, tag=f"yh{g}", name=f"yh{g}")
          for g in range(2)]
    lns = [const.tile([1, T], F32, tag=f"ln{h}", name=f"ln{h}")
           for h in range(NH)]
    F32R = mybir.dt.float32r
    recip = [const.tile([1, T], F32R, tag=f"rcp{h}", name=f"rcp{h}")
             for h in range(NH)]
    ones64 = const.tile([1, 64], F32R, tag="ones64", name="ones64")
    nc.sync.dma_start(ones64[:], dram["one64"][:])
    oex = [None] * NH  # per-head AV psum, allocated in mainpass

    def maxS_block(h, i):
        # one 128-row i-block of the fp16 hi-only max pass; DVE negated
        # row-max into colmax column i
        psb = ps.tile([128, T], F32, tag="big", bufs=3, name="psb")
        for n in range(2):
            nsl = slice(n * 512, (n + 1) * 512)
            nc.tensor.matmul(
                psb[:, nsl],
                lhsT=qm[h][0:64, i * 128:(i + 1) * 128],
                rhs=km[h][0:64, nsl],
                start=True, stop=True,
            )
        nc.vector.reduce_max(colmax[h][:, i:i + 1], psb[:], axis=AX.X,
                             negate=True)

    def trpT(h):
        # (128,8) negated col maxes -> (1,1024) fp16 bias row of qm[h],
        # via 8 single-column stationary matmuls against identity.
        for half in range(2):
            prow = ps.tile([1, 512], F32, tag="big", bufs=3, name="prow")
            for cc in range(4):
                c = half * 4 + cc
                nc.tensor.matmul(
                    prow[:, cc * 128:(cc + 1) * 128],
                    lhsT=colmax[h][:, c:c + 1], rhs=id128[:],
                    start=True, stop=True,
                )
            nc.scalar.copy(qm[h][64:65, half * 512:(half + 1) * 512], prow[:])

    def rest(h):
        # reciprocal broadcast + normalize -> fp16 Wo operand rows
        g, hhalf = divmod(h, 2)
        for n in range(2):
            nsl = slice(n * 512, (n + 1) * 512)
            prec = ps.tile([64, 512], F32, tag="big", bufs=3, name="prec")
            nc.tensor.matmul(prec[:], lhsT=ones64[:], rhs=recip[h][:, nsl],
                             start=True, stop=True)
            nc.vector.tensor_mul(
                yh[g][hhalf * 64:(hhalf + 1) * 64, nsl],
                yun[h][:, nsl], prec[:],
            )

    def mainpass(h, rest_h=None, maxs_h=None, trp_h=None, wo0=False):
        # S^T - max = cross(K=128) + main(K=65 w/ bias row), exp, AV.
        # maxs_h's max-pass blocks are interleaved per j so the PE never
        # idles while DVE drains the reduce stream.
        oex[h] = ps.tile([65, T], F32, tag="av", name="oex")
        at = [None] * 8

        def av(j):
            for n in range(2):
                nsl = slice(n * 512, (n + 1) * 512)
                nc.tensor.matmul(
                    oex[h][:, nsl],
                    lhsT=vpx[j][:, h * (D + 1):(h + 1) * (D + 1)],
                    rhs=at[j][:, nsl],
                    start=(j == 0), stop=(j == 7),
                )

        for j in range(8):
            jsl = slice(j * 128, (j + 1) * 128)
            if maxs_h is not None:
                maxS_block(maxs_h, j)
            psb = ps.tile([128, T], F32, tag="big", bufs=3, name="psb")
            for n in range(2):
                nsl = slice(n * 512, (n + 1) * 512)
                nc.tensor.matmul(
                    psb[:, nsl], lhsT=kc[h][:, jsl], rhs=qc[h][:, nsl],
                    start=True, stop=False,
                )
                nc.tensor.matmul(
                    psb[:, nsl], lhsT=km[h][:, jsl], rhs=qm[h][:, nsl],
                    start=False, stop=True,
                )
            at[j] = atp.tile([128, T], BF16, tag="at", name="at")
            nc.scalar.activation(at[j][:], psb[:], AF.Exp)
            if j >= 2:
                av(j - 2)
            if wo0:
                wo_block(0, j, dve_only=True)
            if j == 3 and rest_h is not None:
                rest(rest_h)
        av(6)
        av(7)
        if trp_h is not None:
            trpT(trp_h)
        # evacuate Y^T (raw) and the rowsums' reciprocal
        nc.vector.tensor_copy(yun[h][:], oex[h][0:64, :])
        nc.scalar.activation(lns[h][:], oex[h][64:65, :], AF.Ln)
        nc.scalar.activation(recip[h][:], lns[h][:], AF.Exp, scale=-1.0)

    def wo_block(g, i, dve_only=False):
        pso = ps.tile([128, E], F32, tag="big", bufs=3, name="pso")
        nc.tensor.matmul(
            pso[:],
            lhsT=yh[g][:, i * 128:(i + 1) * 128],
            rhs=wo[g][:],
            start=True, stop=True,
        )
        o16 = outp.tile([128, E], F16, tag="o16", bufs=4, name="o16")
        if dve_only or i % 2 == 0:
            nc.vector.tensor_copy(o16[:], pso[:])
        else:
            nc.scalar.copy(o16[:], pso[:])
        nc.sync.dma_start(dram[f"out{g}"][i * 128:(i + 1) * 128, :], o16[:])

    def wo_pass(g):
        for i in range(8):
            wo_block(g, i)

    # ---- schedule ----
    emit_vp_maxS0()
    trpT(0)
    mainpass(0, maxs_h=1, trp_h=1)
    mainpass(1, rest_h=0, maxs_h=2, trp_h=2)
    mainpass(2, rest_h=1, maxs_h=3, trp_h=3)
    mainpass(3, rest_h=2, wo0=True)
    # bridge: keep the PE's HAM activity window busy while the rowsum
    # reciprocal chain (Ln+Exp) of head 3 drains on ACT
    for _ in range(10):
        pbr = ps.tile([128, E], F32, tag="big", bufs=3, name="pbr")
        nc.tensor.matmul(pbr[:], lhsT=wrm[:, 0:128], rhs=wrm[:],
                         start=True, stop=True)
    rest(3)
    wo_pass(1)


class _Bacc(bacc.Bacc):
    """Bacc whose activation-table chooser can only pick the combined
    natural_log_exp_and_others set for Exp/Ln (removes per-head Exp<->Ln
    table swaps)."""

    def insert_act_table_loads(self):
        import bass_rust as _bass_rust
        from concourse.hw_specs import get_activation_tables
        has_activation = any(
            isinstance(i, mybir.InstActivation)
            for b in self.main_func.blocks
            for i in b.instructions
        )
        if not has_activation:
            return
        tables = []
        for name, fns in get_activation_tables(self.m.arch).items():
            if name in ("exp_and_others", "exp_and_friends"):
                fns = set()
            tables.append((name, fns))
        _bass_rust.insert_act_table_loads(self, tables)


def build_program():
    nc = _Bacc("TRN2", target_bir_lowering=False, debug=False)
    dp = nc.declare_dram_parameter
    dram = {}
    for name in ("qh", "ql", "kvh", "kvl"):
        dram[name] = dp(name, [128, 4, T], F16, isOutput=False)
    for name in ("wqh", "wql", "wkh", "wkl"):
        dram[name] = dp(name, [128, 4, NH * D], F16, isOutput=False)
    dram["wv"] = dp("wv", [128, 4, DX], F16, isOutput=False)
    dram["wo"] = dp("wo", [128, 2, E], F16, isOutput=False)
    dram["id"] = dp("id", [128, 128], F16, isOutput=False)
    dram["one64"] = dp("one64", [1, 64], mybir.dt.float32r, isOutput=False)
    dram["out0"] = dp("out0", [T, E], F16, isOutput=True)
    dram["out1"] = dp("out1", [T, E], F16, isOutput=True)
    with ExitStack() as ctx:
        tc = ctx.enter_context(tile.TileContext(nc))
        _emit(ctx, tc, dram)
    nc.finalize()
    return nc


_PROGRAM = None


def _get_program():
    global _PROGRAM
    if _PROGRAM is None:
        _PROGRAM = build_program()
    return _PROGRAM


def _split16(x):
    h = x.astype(np.float16)
    l = (x - h.astype(np.float32)).astype(np.float16)
    return h, l


def _pmajor(x, chunks):
    """(C*128, cols) row-major -> (128, C, cols): partition-major layout so
    the input DMA is one contiguous line per partition."""
    rows, cols = x.shape
    return np.ascontiguousarray(
        x.reshape(chunks, 128, cols).transpose(1, 0, 2))


_ID128 = np.eye(128, dtype=np.float16)


def make_in_maps(q, kv, Wq, Wk, Wv, Wo):
    in_maps = []
    for c in range(N_CORES):
        b, g = divmod(c, 2)
        heads = [g * NH + j for j in range(NH)]
        idx_q = [d * H + h for h in heads for d in range(D)]
        idx_k = [((D - d) % D) * H + h for h in heads for d in range(D)]
        qTh, qTl = _split16(np.ascontiguousarray(q[b].T))
        kvTh, kvTl = _split16(np.ascontiguousarray(kv[b].T))
        wq_h, wq_l = _split16(Wq[:, idx_q] * np.float32(SCALE))
        wk_h, wk_l = _split16(Wk[:, idx_k])
        wv_c = np.zeros((E, DX), np.float16)
        for j, h in enumerate(heads):
            wv_c[:, j * (D + 1):j * (D + 1) + D] = \
                Wv[:, [d * H + h for d in range(D)]].astype(np.float16)
        in_maps.append({
            "qh": _pmajor(qTh, 4), "ql": _pmajor(qTl, 4),
            "kvh": _pmajor(kvTh, 4), "kvl": _pmajor(kvTl, 4),
            "wqh": _pmajor(wq_h, 4), "wql": _pmajor(wq_l, 4),
            "wkh": _pmajor(wk_h, 4), "wkl": _pmajor(wk_l, 4),
            "wv": _pmajor(wv_c, 4),
            "wo": _pmajor(
                Wo[g * NH * D:(g + 1) * NH * D, :].astype(np.float16), 2),
            "id": _ID128,
            "one64": np.ones((1, 64), np.float32),
        })
    return in_maps


def kernel(**inputs):
    global LAST_EXEC_NS
    q = np.asarray(inputs["q"], dtype=np.float32)
    kv = np.asarray(inputs["kv"], dtype=np.float32)
    Wq = np.asarray(inputs["Wq"], dtype=np.float32)
    Wk = np.asarray(inputs["Wk"], dtype=np.float32)
    Wv = np.asarray(inputs["Wv"], dtype=np.float32)
    Wo = np.asarray(inputs["Wo"], dtype=np.float32)
    B = q.shape[0]

    nc = _get_program()
    in_maps = make_in_maps(q, kv, Wq, Wk, Wv, Wo)
    res = run_bass_kernel_spmd(nc, in_maps, list(range(N_CORES)), trace=TRACE)
    LAST_EXEC_NS = res.exec_time_ns

    out = np.empty((B, T, E), np.float32)
    for b in range(B):
        out[b] = (res.results[2 * b]["out0"].astype(np.float32)
                  + res.results[2 * b]["out1"].astype(np.float32)
                  + res.results[2 * b + 1]["out0"].astype(np.float32)
                  + res.results[2 * b + 1]["out1"].astype(np.float32))
    return out
